# revision 23
# baseline (speedup 1.0000x reference)
"""Trainium2 Bass kernel for nn_FNO1DDecoder (dense_mlp).

Math: the reference is
    h   = token @ w_dec + b_dec                  # [B, 2048]
    modes -> zero-padded spectrum -> irfft(L=8192)  # [B, 64, 8192]
    x   = irfft[..., :-2].T                      # [B, 8190, 64]
    y   = gelu(x @ w1 + b1) @ w2 + b2            # [B, 8190, 1]

Since the irfft of a 16-mode spectrum is a linear map with a fixed
cos/sin basis F [L, 32] (x[b, n, w] = sum_k F[n, k] h2[b, w, k]), and
fc1 is linear, we fold w1 through it:
    g[b, k, j]   = sum_w h2[b, w, k] w1[w, j]    # [B, 32, 128]  (tiny)
    pre1[b,n,j]  = sum_k F[n, k] g[b, k, j]
    y[b, n]      = sum_j w2[j] gelu(pre1 + b1) + b2
This cuts FLOPs ~2.4x and removes the giant irfft entirely.

Sharding: pure data parallel over batch (8 batches per core), weights
replicated.  The F matmul uses 4-way PE row tiling (K=32 per tile).
Cos/sin symmetry (F[L-n, cos] = F[n, cos], F[L-n, sin] = -F[n, sin])
halves the basis: the back half of the spectrum is computed with a
sign-flipped g against the same F columns; the host un-reverses it.

The w2 contraction runs as act-stationary matmuls (out = [128 n, 1]),
keeping all matmul PSUM outputs at partition 0 (ISA requirement) and
making y evacuation a dense [128, 16] DVE copy per slot pair.

bf16 is used for the big DMAs (w_dec, F) and matmul operands feeding
f32-PSUM accumulations; activations are fp16 for the w2 stage.
"""

import numpy as np
import ml_dtypes

from concourse import bacc, bass, mybir, tile
from concourse.bass_utils import run_bass_kernel_spmd

F32 = mybir.dt.float32
BF16 = mybir.dt.bfloat16
F16 = mybir.dt.float16
F8 = mybir.dt.float8e4
GELU = mybir.ActivationFunctionType.Gelu

B, EMB, FDIM, W, K32, J, L = 64, 1024, 2048, 64, 32, 128, 8192
NCORES, BPC = 8, 8          # batches per core
NT = 512                    # n-tile width
HALF_TILES = 8              # tiles per half (front n in [0,4096), back m in [1,4096])


def build_program():
    nc = bacc.Bacc("TRN2", target_bir_lowering=False, debug=False)

    # token arranged on host to [p, (e b)] so the DMA is contiguous
    tokA = nc.dram_tensor("tokA", [128, 64], BF16, kind="ExternalInput").ap()
    wdec = nc.dram_tensor("wdec", [EMB, FDIM], BF16, kind="ExternalInput").ap()
    bdecr = nc.dram_tensor("bdecr", [W, BPC * K32], BF16, kind="ExternalInput").ap()
    w1 = nc.dram_tensor("w1", [W, J], BF16, kind="ExternalInput").ap()
    b1c = nc.dram_tensor("b1c", [J, 1], F32, kind="ExternalInput").ap()
    w2c = nc.dram_tensor("w2c", [J, 1], F16, kind="ExternalInput").ap()
    b2c = nc.dram_tensor("b2c", [J, 1], F32, kind="ExternalInput").ap()
    fbas = nc.dram_tensor("fbas", [128, 4097], BF16, kind="ExternalInput").ap()
    ident = nc.dram_tensor("ident", [128, 128], F32, kind="ExternalInput").ap()
    sgn = nc.dram_tensor("sgn", [128, 1], F32, kind="ExternalInput").ap()
    out = nc.dram_tensor("out", [128, NT], F32, kind="ExternalOutput").ap()
    h2scr = nc.dram_tensor("h2scr", [BPC, FDIM], BF16).ap()

    with tile.TileContext(nc) as tc:
        with tc.tile_pool(name="consts", bufs=1) as cp:
            # small consts on the scalar (ACT) HWDGE ring
            tokT_sb = cp.tile([128, 64], BF16)
            nc.scalar.dma_start(tokT_sb[:], tokA)
            w1_sb = cp.tile([W, J], BF16)
            nc.scalar.dma_start(w1_sb[:], w1)
            b1_sb = cp.tile([J, 1], F32)
            nc.scalar.dma_start(b1_sb[:], b1c)
            w2_sb = cp.tile([J, 1], F16)
            nc.scalar.dma_start(w2_sb[:], w2c)
            b2_sb = cp.tile([J, 1], F32)
            nc.scalar.dma_start(b2_sb[:], b2c)
            sgn_sb = cp.tile([128, 1], F32)
            nc.scalar.dma_start(sgn_sb[:], sgn)
            bdecr_sb = cp.tile([W, BPC * K32], BF16)
            nc.scalar.dma_start(bdecr_sb[:], bdecr)
            id_sb = cp.tile([128, 128], F32)
            nc.scalar.dma_start(id_sb[:], ident)

            g_st = cp.tile([128, 256], BF16)   # cols [128*grp:...] = stationaries
            g_stb = cp.tile([128, 256], BF16)  # sign-flipped (back half)
            h2r_sb = cp.tile([W, BPC * K32], BF16)
            fb_sb = cp.tile([128, 4097], BF16)

            # ---- decode head: h2 = token @ w_dec (+ b_dec later) ----
            # w_dec chunks stream on the sync HWDGE ring (dedicated)
            with (
                tc.tile_pool(name="decps", bufs=1, space="PSUM") as dps,
                tc.tile_pool(name="wdecp", bufs=8) as wp,
            ):
                h2_ps = dps.tile([BPC, FDIM], F32)
                wts = []
                for ei in range(8):
                    wt = wp.tile([128, FDIM], BF16)
                    eng = nc.sync if ei % 2 == 0 else nc.scalar
                    eng.dma_start(wt[:], wdec[128 * ei:128 * (ei + 1), :])
                    wts.append(wt)
                for ei in range(8):
                    for nf in range(4):
                        nc.tensor.matmul(
                            h2_ps[:, NT * nf:NT * (nf + 1)],
                            tokT_sb[:, 8 * ei:8 * ei + 8],
                            wts[ei][:, NT * nf:NT * (nf + 1)],
                            start=(ei == 0), stop=(ei == 7),
                        )
                # F basis: big const, scalar ring, needed only at main loop
                nc.scalar.dma_start(fb_sb[:], fbas)

                # rearrange [b, (w k)] -> [w, (b k)] via DRAM bounce
                h2_sb = cp.tile([128, FDIM], BF16)
                nc.vector.tensor_copy(h2_sb[:BPC, :], h2_ps[:])
                nc.sync.dma_start(h2scr, h2_sb[:BPC, :])
                nc.sync.dma_start(
                    h2r_sb[:].rearrange("w (b k) -> w b k", b=BPC),
                    h2scr.rearrange("b (w k) -> w b k", w=W),
                )
                with nc.allow_low_precision(reason="bf16 h2 + b_dec add"):
                    nc.vector.tensor_add(h2r_sb[:], h2r_sb[:], bdecr_sb[:])

                # ---- g = w1.T-contract: gT [j, (b k)] ----
                g_ps = dps.tile([J, BPC * K32], F32)
                nc.tensor.matmul(
                    g_ps[:], w1_sb[:], h2r_sb[:],
                    start=True, stop=True,
                )
                gT_sb = cp.tile([J, BPC * K32], F32)
                nc.vector.tensor_copy(gT_sb[:], g_ps[:])
                # transpose 128-col blocks -> row-tiled stationaries
                for grp in range(2):
                    t_ps = dps.tile([128, 128], F32)
                    nc.tensor.matmul(
                        t_ps[:], gT_sb[:, 128 * grp:128 * (grp + 1)], id_sb[:],
                        is_transpose=True, start=True, stop=True,
                    )
                    nc.vector.tensor_copy(g_st[:, 128 * grp:128 * (grp + 1)], t_ps[:])
                    nc.vector.tensor_scalar_mul(
                        g_stb[:, 128 * grp:128 * (grp + 1)],
                        t_ps[:], sgn_sb[:, 0:1],
                    )

            # ---- main loop ----
            # Slot = one grp (4 batches) x one 512-col n-tile: 4-way row
            # tiling fills the PE; ACT runs one [128, 2048] instr per slot.
            # y matmuls for slot s run one slot later (TensorE never waits
            # on ACT), writing into slot s's own just-freed PSUM bank 0.
            ybuf = cp.tile([128, 512], F32)
            with (
                tc.tile_pool(name="pre1", bufs=2, space="PSUM") as pp,
                tc.tile_pool(name="acts", bufs=3) as ap_,
            ):
                slots = []
                for grp in range(2):
                    for half in range(2):
                        for t in range(HALF_TILES):
                            slots.append((grp, half, t))

                pending = []  # (slot_tile, act_t, slot_idx)

                def flush_pending():
                    slot_p, act_p, si_p = pending.pop(0)
                    for q in range(4):
                        for cc in range(4):
                            nc.tensor.matmul(
                                slot_p[:, 4 * q + cc:4 * q + cc + 1],
                                act_p[:, NT * q + 128 * cc:
                                      NT * q + 128 * (cc + 1)],
                                w2_sb[:],
                                start=True, stop=True,
                            )
                    nc.vector.tensor_copy(
                        ybuf[:, 16 * si_p:16 * (si_p + 1)], slot_p[:, 0:16]
                    )

                for si, (grp, half, t) in enumerate(slots):
                    col0 = NT * t if half == 0 else 3585 - NT * t
                    gsrc = g_st if half == 0 else g_stb
                    slot = pp.tile([128, 4 * NT], F32)
                    for q in range(4):
                        rb = 32 * q
                        nc.tensor.matmul(
                            slot[:, NT * q:NT * (q + 1)],
                            gsrc[rb:rb + 32, 128 * grp:128 * (grp + 1)],
                            fb_sb[rb:rb + 32, col0:col0 + NT],
                            start=True, stop=True,
                            tile_position=(rb, 0),
                        )
                    act_t = ap_.tile([128, 4 * NT], F16)
                    nc.scalar.activation(
                        act_t[:], slot[:], GELU, bias=b1_sb[:, 0:1]
                    )
                    pending.append((slot, act_t, si))
                    if len(pending) > 1:
                        flush_pending()
                while pending:
                    flush_pending()
                nc.vector.tensor_scalar_add(ybuf[:], ybuf[:], b2_sb[:, 0:1])
                nc.sync.dma_start(out, ybuf[:])
    nc.compile()
    return nc


def host_inputs(token, w_dec, b_dec, w1, b1, w2, b2):
    """Build the per-core input maps (host-side data movement only)."""
    token = np.ascontiguousarray(np.asarray(token, np.float32))
    w_dec = np.ascontiguousarray(np.asarray(w_dec, np.float32))
    b_dec = np.asarray(b_dec, np.float32)
    w1 = np.ascontiguousarray(np.asarray(w1, np.float32))
    b1 = np.asarray(b1, np.float32)
    w2 = np.asarray(w2, np.float32)
    b2 = np.asarray(b2, np.float32)

    c = np.arange(4097)[None, :]
    m = np.arange(16)[:, None]
    ang = 2.0 * np.pi * m * c / L
    base = np.empty((32, 4097), np.float32)
    base[0::2] = (2.0 / L) * np.cos(ang)
    base[1::2] = -(2.0 / L) * np.sin(ang)
    base[0] = 1.0 / L
    base[1] = 0.0
    fbas = np.ascontiguousarray(np.tile(base, (4, 1))).astype(ml_dtypes.bfloat16)

    bdecr = np.ascontiguousarray(
        np.tile(b_dec.reshape(W, 1, K32), (1, BPC, 1)).reshape(W, BPC * K32)
    ).astype(ml_dtypes.bfloat16)
    sgn = np.where((np.arange(128) % 32) % 2 == 1, -1.0, 1.0).astype(np.float32)

    common = dict(
        wdec=np.ascontiguousarray(w_dec).astype(ml_dtypes.bfloat16),
        bdecr=bdecr,
        w1=np.ascontiguousarray(w1).astype(ml_dtypes.bfloat16),
        b1c=np.ascontiguousarray(b1.reshape(J, 1)),
        w2c=np.ascontiguousarray(w2.reshape(J, 1).astype(np.float16)),
        b2c=np.full((J, 1), float(b2.reshape(-1)[0]), np.float32),
        fbas=fbas,
        ident=np.eye(128, dtype=np.float32),
        sgn=np.ascontiguousarray(sgn.reshape(128, 1)),
    )
    in_maps = []
    for core in range(NCORES):
        m_ = dict(common)
        # [p, (e b)]: tokA[p, 8e+b] = token[8 core + b, 128 e + p]
        sl = token[BPC * core:BPC * (core + 1), :]           # [8, 1024]
        tokA = sl.reshape(BPC, 8, 128).transpose(2, 1, 0)    # [p, e, b]
        m_["tokA"] = np.ascontiguousarray(tokA.reshape(128, 64)).astype(
            ml_dtypes.bfloat16)
        in_maps.append(m_)
    return in_maps


def assemble_output(raws):
    """raws: list of 8 per-core [128, 512] arrays -> [64, 8190, 1].

    Raw layout: raw[p, 16*si + 4*q + cc] = y[b, n] with si enumerating
    (grp, half, t); b = 4*grp + q (+ 8*core); n = 512*t + 128*cc + p
    (front half) or n = 8192 - m with m = 3585 - 512*t + 128*cc + p
    (back half, m <= 8189 kept).
    """
    y = np.empty((B, L - 2), np.float32)
    p = np.arange(128)
    for core in range(NCORES):
        raw = np.asarray(raws[core])
        si = 0
        for grp in range(2):
            for half in range(2):
                for t in range(HALF_TILES):
                    for q in range(4):
                        b = BPC * core + 4 * grp + q
                        for cc in range(4):
                            col = raw[:, 16 * si + 4 * q + cc]
                            if half == 0:
                                n0 = NT * t + 128 * cc
                                y[b, n0:n0 + 128] = col
                            else:
                                m = 3585 - NT * t + 128 * cc + p
                                n = L - m
                                valid = n <= L - 3
                                y[b, n[valid]] = col[valid]
                    si += 1
    return y.reshape(B, L - 2, 1)


_NC_CACHE = None


def kernel(token, x_len, w_dec, b_dec, w1, b1, w2, b2):
    global _NC_CACHE
    assert int(x_len) == L, f"kernel hardcodes x_len={L}, got {x_len}"
    if _NC_CACHE is None:
        _NC_CACHE = build_program()
    nc = _NC_CACHE
    in_maps = host_inputs(token, w_dec, b_dec, w1, b1, w2, b2)
    res = run_bass_kernel_spmd(nc, in_maps, core_ids=list(range(NCORES)))
    return assemble_output([res.results[i]["out"] for i in range(NCORES)])
